# revision 24
# baseline (speedup 1.0000x reference)
"""LSTM (BaseRNN) Trainium2 kernel.

Problem: B=128, T=512, I=256, H=768 LSTM; returns (hiddenStates, cellStates)
each [B, T, H] fp32.

Strategy (data-parallel over batch, 8 cores x 16 rows):
  - Batch-major gate preactivations g_t = x_t W + h_{t-1} U accumulated in
    PSUM as [batch, gate_cols]; stationary operand = x^T / h^T chunks
    [128, 16] bf16, moving operand = W/U chunks [128, 384] bf16.
  - 4-way PE col-group tiling: gate q -> col group q (psum partitions
    32q..); the four groups' matmuls run concurrently in the array.
  - The step is processed in two independent column halves with SEPARATE
    psum tiles so each half's activation chain starts as soon as its own
    32 matmuls finish (per-tile dependency tracking).
  - Software pipelining: the x-only waves (k<KX) of step t+1 are emitted
    before step t's transposes, so the PE has independent work during the
    elementwise tail and the HAM clock gate stays warm.
  - Host permutes gate columns to (i, f, o, g~).  One ACT sigmoid with a
    per-partition scale vector (1 for i/f/o rows, 2 for g~ rows) covers all
    four gates; tanh(x) = 2*sigmoid(2x)-1 is reconstructed by a cheap DVE
    tensor_scalar.  All 16-bit tensors are bf16 (DVE 2x packed mode).
  - h = o' * tanh(c) computed in normal space (1 DVE op), then 3 PE
    transposes + 1 DVE psum->sbuf copy produce the h^T stationary.
  - hs/cs stream to DRAM as bf16; the host upcasts to fp32.
"""

import numpy as np
import ml_dtypes

import concourse.bass as bass
import concourse.bacc as bacc
import concourse.tile as tile
from concourse import mybir
from concourse.bass_utils import run_bass_kernel_spmd
from concourse.masks import make_identity

B, T, I, H = 128, 512, 256, 768
NCORES = 8
NB = B // NCORES  # 16
KX = I // 128  # 2 x chunks
KH = H // 128  # 6 h chunks
NK = KX + KH  # 8 contraction waves
NHALF = H // 2  # 384: per-gate psum half (one matmul's N)
F32 = mybir.dt.float32
BF16 = mybir.dt.bfloat16
XBLK = 32  # x-stream block (steps per DMA)

MM_DT = BF16  # matmul operand dtype


def build_lstm(nb=NB, t_steps=T, has_b=False):
    nc = bacc.Bacc(None, target_bir_lowering=False)

    xT_d = nc.dram_tensor("xT", [t_steps, KX, 128, nb], F32, kind="ExternalInput")
    h0_d = nc.dram_tensor("h0", [nb, H], F32, kind="ExternalInput")
    c0_d = nc.dram_tensor("c0", [nb, 2, NHALF], BF16, kind="ExternalInput")
    w_d = nc.dram_tensor("w", [KX, 128, 4 * H], F32, kind="ExternalInput")
    u_d = nc.dram_tensor("u", [KH, 128, 4 * H], F32, kind="ExternalInput")
    b_d = nc.dram_tensor("b", [1, 4 * H], F32, kind="ExternalInput")
    hs_d = nc.dram_tensor("hs", [t_steps, 128, KH * NB], BF16, kind="ExternalOutput")
    cs_d = nc.dram_tensor("cs", [nb, t_steps, 2, NHALF], BF16, kind="ExternalOutput")

    SIG = mybir.ActivationFunctionType.Sigmoid
    TANH = mybir.ActivationFunctionType.Tanh
    MULT = mybir.AluOpType.mult
    ADD = mybir.AluOpType.add

    with tile.TileContext(nc) as tc:
        with (
            tc.tile_pool(name="consts", bufs=1) as consts,
            tc.tile_pool(name="xs", bufs=2) as xs_pool,
            tc.tile_pool(name="gsb", bufs=3) as gsb,
            tc.tile_pool(name="ew", bufs=3) as ew,
            tc.tile_pool(name="state", bufs=2) as state,
            tc.tile_pool(name="pg", bufs=2, space="PSUM") as pg,
            tc.tile_pool(name="pt", bufs=2, space="PSUM") as pt,
        ):
            ident16 = consts.tile([nb, nb], BF16)
            make_identity(nc, ident16)

            # weights: DMA f32 staging -> round-convert to bf16
            w_sb = consts.tile([128, KX, 4 * H], MM_DT)
            u_sb = consts.tile([128, KH, 4 * H], MM_DT)
            for k in range(KX):
                stg = xs_pool.tile([128, 4 * H], F32, tag="WSTG")
                nc.sync.dma_start(out=stg, in_=w_d[k, :, :])
                nc.vector.tensor_copy(w_sb[:, k, :], stg)
            for k in range(KH):
                stg = xs_pool.tile([128, 4 * H], F32, tag="WSTG")
                nc.sync.dma_start(out=stg, in_=u_d[k, :, :])
                nc.vector.tensor_copy(u_sb[:, k, :], stg)
            if has_b:
                b_sb = consts.tile([1, 4 * H], F32)
                nc.sync.dma_start(out=b_sb, in_=b_d[:, :])

            # ---- initial state ----
            h0_sb = consts.tile([nb, H], F32)
            nc.sync.dma_start(out=h0_sb, in_=h0_d[:, :])
            c_prev = [None, None]
            for n2 in range(2):
                ct = state.tile([32 + nb, NHALF], BF16, tag=f"C{n2}")
                nc.sync.dma_start(out=ct[32 : 32 + nb], in_=c0_d[:, n2, :])
                c_prev[n2] = ct

            h0_16 = consts.tile([nb, H], BF16)
            nc.vector.tensor_copy(h0_16, h0_sb)
            ht_prev = state.tile([128, KH * nb], MM_DT, tag="HT")
            for n2 in range(2):
                ht0_ps = pt.tile([128, 3 * nb], BF16, tag="hTps")
                for j in range(3):
                    ck = 3 * n2 + j
                    nc.tensor.transpose(
                        ht0_ps[:, j * nb : (j + 1) * nb],
                        h0_16[:, ck * 128 : (ck + 1) * 128],
                        ident16,
                    )
                nc.scalar.copy(
                    out=ht_prev[:, 3 * n2 * nb : (3 * n2 + 3) * nb], in_=ht0_ps
                )

            def stage_x_dma(t0):
                """Issue the DMA for the x block starting at step t0."""
                xstg = xs_pool.tile([128, XBLK, KX, nb], F32, tag="XSTG", name="xstg")
                nblk = min(XBLK, t_steps - t0)
                nc.sync.dma_start(
                    out=xstg[:, 0:nblk],
                    in_=xT_d[t0 : t0 + nblk].rearrange("t k p b -> p t k b"),
                )
                return xstg, nblk

            def convert_x(xstg, nblk):
                """bf16-convert a staged x block (DMA issued well before)."""
                xt = xs_pool.tile([128, XBLK, KX, nb], MM_DT, tag="X", name="xt")
                nc.vector.tensor_copy(xt[:, 0:nblk], xstg[:, 0:nblk])
                return xt

            def emit_x_waves(gates2, x_tile, t):
                """x-only matmul waves (k < KX) for step t, both halves."""
                trel = t % XBLK
                for n2 in range(2):
                    for k in range(KX):
                        for q in range(4):
                            col = q * H + n2 * NHALF
                            nc.tensor.matmul(
                                gates2[n2][32 * q : 32 * q + nb, 0:NHALF],
                                x_tile[:, trel, k, :],
                                w_sb[:, k, col : col + NHALF],
                                start=(k == 0),
                                stop=False,
                                tile_position=(0, 32 * q),
                                skip_group_check=True,
                            )

            def emit_h_waves(gates2, ht, n2):
                """h matmul waves (k in [KX, NK)) for one half."""
                for k in range(KX, NK):
                    ck = k - KX
                    for q in range(4):
                        col = q * H + n2 * NHALF
                        nc.tensor.matmul(
                            gates2[n2][32 * q : 32 * q + nb, 0:NHALF],
                            ht[:, ck * nb : (ck + 1) * nb],
                            u_sb[:, ck, col : col + NHALF],
                            start=False,
                            stop=(k == NK - 1),
                            tile_position=(0, 32 * q),
                            skip_group_check=True,
                        )

            def new_gates():
                return [
                    pg.tile([128, 512], F32, tag="g0", name="gates0", bufs=3),
                    pg.tile([128, 512], F32, tag="g1", name="gates1", bufs=3),
                ]

            # prologue: stage x block 0; emit x-waves for steps 0 and 1
            # (x-waves run two steps ahead so the PE always has
            # recurrence-independent work during the elementwise tail)
            x_tile = convert_x(*stage_x_dma(0))
            gates_cur = new_gates()
            emit_x_waves(gates_cur, x_tile, 0)
            gates_next = new_gates()
            emit_x_waves(gates_next, x_tile, 1)

            for t in range(t_steps):
                # ---- PE: h-waves for this step (both halves) ----
                for n2 in range(2):
                    emit_h_waves(gates_cur, ht_prev, n2)

                gates = gates_cur
                if has_b:
                    for n2 in range(2):
                        for q, base in ((0, 0), (1, 32), (2, 64), (3, 96)):
                            bq = b_sb[:, q * H + n2 * NHALF : q * H + (n2 + 1) * NHALF]
                            bq = bass.AP(
                                tensor=bq.tensor, offset=bq.offset,
                                ap=[[0, nb]] + bq.ap[1:],
                            )
                            nc.vector.tensor_add(
                                gates[n2][base : base + nb, 0:NHALF],
                                gates[n2][base : base + nb, 0:NHALF],
                                bq,
                            )

                # ---- PE: x-waves for step t+2 (independent of h) ----
                gates_cur = gates_next
                if t + 2 < t_steps:
                    # prefetch next block's DMA at mid-block; convert at block edge
                    if (t + 2) % XBLK == XBLK // 2 and t + 2 + XBLK // 2 < t_steps:
                        xstg_pend = stage_x_dma(t + 2 + XBLK // 2)
                    if (t + 2) % XBLK == 0:
                        x_tile = convert_x(*xstg_pend)
                    gates_next = new_gates()
                    emit_x_waves(gates_next, x_tile, t + 2)

                # ---- ACT: sigmoids (one per half; per-row scale does g~) ----
                # S2 rows: i'@0:16, f'@32:48, o'@64:80, sigmoid(2g)@96:112
                S2, C, G, TC, HN = {}, {}, {}, {}, {}
                # (g~ weight columns are pre-doubled on the host, so a plain
                # sigmoid gives sigmoid(2g) on those rows)
                for n2 in range(2):
                    S2[n2] = gsb.tile([112, NHALF], BF16, tag=f"S2{n2}", name=f"S2_{n2}")
                    nc.scalar.activation(
                        out=S2[n2], in_=gates[n2][0:112, 0:NHALF], func=SIG,
                    )

                # ---- DVE chains (half A fully, then half B) ----
                for n2 in range(2):
                    # g~ = 2*sigmoid(2g) - 1  (rows 0:16, matching i' rows)
                    G[n2] = gsb.tile([nb, NHALF], BF16, tag=f"G{n2}", name=f"G_{n2}")
                    nc.vector.tensor_scalar(
                        G[n2], S2[n2][96:112], 2.0, -1.0, MULT, ADD
                    )
                    # i' * g~ (out-shift to rows 32:48 so the add's inputs align)
                    T1 = ew.tile([32 + nb, NHALF], BF16, tag=f"T1{n2}", name=f"T1_{n2}")
                    nc.vector.tensor_mul(T1[32 : 32 + nb], S2[n2][0:nb], G[n2])
                    # f' * c
                    C[n2] = state.tile(
                        [32 + nb, NHALF], BF16, tag=f"C{n2}", name=f"C_{n2}"
                    )
                    nc.vector.tensor_mul(
                        C[n2][32 : 32 + nb], S2[n2][32 : 32 + nb],
                        c_prev[n2][32 : 32 + nb],
                    )
                    nc.vector.tensor_add(
                        C[n2][32 : 32 + nb], C[n2][32 : 32 + nb],
                        T1[32 : 32 + nb],
                    )

                # ---- per half: tanh(c) -> h = o'*tanh(c) -> transpose ->
                # psum->sbuf copy on ACT (DVE stays free for the other
                # half's chain; per-half h^T tiles so next-step waves k=2-4
                # only wait on the A copy) ----
                ht_new = state.tile([128, KH * nb], MM_DT, tag="HT", name="ht_new")
                for n2 in range(2):
                    TC[n2] = ew.tile(
                        [64 + nb, NHALF], BF16, tag=f"TC{n2}", name=f"TC_{n2}"
                    )
                    nc.scalar.activation(
                        out=TC[n2][64 : 64 + nb], in_=C[n2][32 : 32 + nb], func=TANH,
                    )
                    HN[n2] = ew.tile([nb, NHALF], BF16, tag=f"H{n2}", name=f"H_{n2}")
                    nc.vector.tensor_mul(
                        HN[n2], S2[n2][64 : 64 + nb], TC[n2][64 : 64 + nb]
                    )
                    hT_ps = pt.tile([128, 3 * nb], BF16, tag="hTps", name="hT_ps")
                    for j in range(3):
                        nc.tensor.transpose(
                            hT_ps[:, j * nb : (j + 1) * nb],
                            HN[n2][:, j * 128 : (j + 1) * 128],
                            ident16,
                        )
                    # this copy gates the next step's h-waves — boost its
                    # priority so the scheduler runs it ahead of the other
                    # half's chain ops in the DVE queue
                    with tc.high_priority():
                        nc.vector.tensor_copy(
                            ht_new[:, 3 * n2 * nb : (3 * n2 + 3) * nb], hT_ps
                        )

                for n2 in range(2):
                    nc.sync.dma_start(
                        out=cs_d[:, t, n2], in_=C[n2][32 : 32 + nb]
                    )
                    c_prev[n2] = C[n2]
                nc.sync.dma_start(out=hs_d[t, :, :], in_=ht_new)

                ht_prev = ht_new

    nc.finalize()
    return nc


# Column permutation: reference gate order (i, f, g~, o) -> kernel (i, f, o, g~)
def _gate_perm():
    return np.concatenate(
        [np.arange(0, H), np.arange(H, 2 * H), np.arange(3 * H, 4 * H),
         np.arange(2 * H, 3 * H)]
    )


def _prep_core_inputs(input_, h0, c0, Wp, Up, bp, t_steps):
    nb = input_.shape[0]
    xT = np.ascontiguousarray(
        input_[:, :t_steps].transpose(1, 2, 0).reshape(t_steps, KX, 128, nb)
    )
    return {
        "xT": xT,
        "h0": np.ascontiguousarray(h0),
        "c0": np.ascontiguousarray(
            c0.reshape(nb, 2, NHALF).astype(ml_dtypes.bfloat16)
        ),
        "w": Wp,
        "u": Up,
        "b": bp,
    }


def run(input, hiddenState, cellState, W, U, b, t_steps=T, trace=False):
    input = np.asarray(input, np.float32)
    hiddenState = np.asarray(hiddenState, np.float32)
    cellState = np.asarray(cellState, np.float32)
    W = np.asarray(W, np.float32)
    U = np.asarray(U, np.float32)
    b = np.asarray(b, np.float32)

    perm = _gate_perm()
    # pre-double the g~ gate columns (kernel gate order (i,f,o,g~) -> cols
    # 3H:4H) so the device sigmoid computes sigmoid(2g) with no scale input
    Wp = W[:, perm].copy()
    Up = U[:, perm].copy()
    bp = b[perm].copy()
    Wp[:, 3 * H :] *= 2.0
    Up[:, 3 * H :] *= 2.0
    bp[3 * H :] *= 2.0
    Wp = np.ascontiguousarray(Wp.reshape(KX, 128, 4 * H))
    Up = np.ascontiguousarray(Up.reshape(KH, 128, 4 * H))
    bp = np.ascontiguousarray(bp.reshape(1, 4 * H))
    has_b = bool(np.any(b))

    nc = build_lstm(NB, t_steps, has_b)
    in_maps = []
    for c in range(NCORES):
        bs = slice(c * NB, (c + 1) * NB)
        in_maps.append(
            _prep_core_inputs(
                input[bs], hiddenState[bs], cellState[bs], Wp, Up, bp, t_steps
            )
        )
    res = run_bass_kernel_spmd(
        nc, in_maps, core_ids=list(range(NCORES)), trace=trace
    )

    hs = np.empty((B, t_steps, H), np.float32)
    cs = np.empty((B, t_steps, H), np.float32)
    for c in range(NCORES):
        bs = slice(c * NB, (c + 1) * NB)
        ht = res.results[c]["hs"].astype(np.float32)  # [t, 128, 6*16]
        ht = ht.reshape(t_steps, 128, KH, NB)
        hs[bs] = ht.transpose(3, 0, 2, 1).reshape(NB, t_steps, H)
        cs[bs] = res.results[c]["cs"].astype(np.float32).reshape(NB, t_steps, H)
    return (hs, cs), res


def kernel(input, hiddenState, cellState, W, U, b):
    (hs, cs), _ = run(input, hiddenState, cellState, W, U, b)
    return hs, cs


# revision 26
# speedup vs baseline: 1.1961x; 1.1961x over previous
"""LSTM (BaseRNN) Trainium2 kernel.

Problem: B=128, T=512, I=256, H=768 LSTM; returns (hiddenStates, cellStates)
each [B, T, H] fp32.

Strategy (data-parallel over batch, 8 cores x 16 rows):
  - Batch-major gate preactivations g_t = x_t W + h_{t-1} U accumulated in
    PSUM as [batch, gate_cols]; stationary operand = x^T / h^T chunks
    [128, 16] bf16, moving operand = W/U chunks [128, 384] bf16.
  - 4-way PE col-group tiling: gate q -> col group q (psum partitions
    32q..); the four groups' matmuls run concurrently in the array.
  - The step is processed in two independent column halves with SEPARATE
    psum tiles so each half's activation chain starts as soon as its own
    32 matmuls finish (per-tile dependency tracking).
  - Software pipelining: the x-only waves (k<KX) run two steps ahead of
    the recurrence, so the PE has independent work during the elementwise
    tail.
  - Host permutes gate columns to (i, f, o, g~) and pre-doubles the g~
    weight columns.  One plain ACT sigmoid covers all four gates (giving
    sigmoid(2g) on the g~ rows); tanh(g) = 2*sigmoid(2g)-1 is
    reconstructed by a cheap DVE tensor_scalar.  All 16-bit tensors are
    bf16.
  - h = o' * tanh(c) computed in normal space (1 DVE op), then 3 PE
    transposes + 1 DVE psum->sbuf copy produce the h^T stationary.
  - hs/cs stream to DRAM as bf16; the host upcasts to fp32.
"""

import numpy as np
import ml_dtypes

import concourse.bass as bass
import concourse.bacc as bacc
import concourse.tile as tile
from concourse import mybir
from concourse.bass_utils import run_bass_kernel_spmd
from concourse.masks import make_identity

B, T, I, H = 128, 512, 256, 768
NCORES = 8
NB = B // NCORES  # 16
KX = I // 128  # 2 x chunks
KH = H // 128  # 6 h chunks
NK = KX + KH  # 8 contraction waves
NHALF = H // 2  # 384: per-gate psum half (one matmul's N)
F32 = mybir.dt.float32
BF16 = mybir.dt.bfloat16
XBLK = 32  # x-stream block (steps per DMA)

MM_DT = BF16  # matmul operand dtype


def build_lstm(nb=NB, t_steps=T, has_b=False):
    nc = bacc.Bacc(None, target_bir_lowering=False)

    xT_d = nc.dram_tensor("xT", [t_steps, KX, 128, nb], F32, kind="ExternalInput")
    h0_d = nc.dram_tensor("h0", [nb, H], F32, kind="ExternalInput")
    c0_d = nc.dram_tensor("c0", [nb, 2, NHALF], BF16, kind="ExternalInput")
    w_d = nc.dram_tensor("w", [KX, 128, 4 * H], F32, kind="ExternalInput")
    u_d = nc.dram_tensor("u", [KH, 128, 4 * H], F32, kind="ExternalInput")
    b_d = nc.dram_tensor("b", [1, 4 * H], F32, kind="ExternalInput")
    hs_d = nc.dram_tensor("hs", [t_steps, 128, KH * NB], BF16, kind="ExternalOutput")
    cs_d = nc.dram_tensor("cs", [nb, t_steps, 2, NHALF], BF16, kind="ExternalOutput")

    SIG = mybir.ActivationFunctionType.Sigmoid
    TANH = mybir.ActivationFunctionType.Tanh
    MULT = mybir.AluOpType.mult
    ADD = mybir.AluOpType.add

    with tile.TileContext(nc) as tc:
        with (
            tc.tile_pool(name="consts", bufs=1) as consts,
            tc.tile_pool(name="xs", bufs=2) as xs_pool,
            tc.tile_pool(name="gsb", bufs=3) as gsb,
            tc.tile_pool(name="ew", bufs=3) as ew,
            tc.tile_pool(name="state", bufs=2) as state,
            tc.tile_pool(name="pg", bufs=2, space="PSUM") as pg,
            tc.tile_pool(name="pt", bufs=2, space="PSUM") as pt,
        ):
            ident16 = consts.tile([nb, nb], BF16)
            make_identity(nc, ident16)

            # weights: DMA f32 staging -> round-convert to bf16
            w_sb = consts.tile([128, KX, 4 * H], MM_DT)
            u_sb = consts.tile([128, KH, 4 * H], MM_DT)
            for k in range(KX):
                stg = xs_pool.tile([128, 4 * H], F32, tag="WSTG")
                nc.sync.dma_start(out=stg, in_=w_d[k, :, :])
                nc.vector.tensor_copy(w_sb[:, k, :], stg)
            for k in range(KH):
                stg = xs_pool.tile([128, 4 * H], F32, tag="WSTG")
                nc.sync.dma_start(out=stg, in_=u_d[k, :, :])
                nc.vector.tensor_copy(u_sb[:, k, :], stg)
            if has_b:
                b_sb = consts.tile([1, 4 * H], F32)
                nc.sync.dma_start(out=b_sb, in_=b_d[:, :])

            # ---- initial state ----
            h0_sb = consts.tile([nb, H], F32)
            nc.sync.dma_start(out=h0_sb, in_=h0_d[:, :])
            c_prev = [None, None]
            for n2 in range(2):
                ct = state.tile([32 + nb, NHALF], BF16, tag=f"C{n2}")
                nc.sync.dma_start(out=ct[32 : 32 + nb], in_=c0_d[:, n2, :])
                c_prev[n2] = ct

            h0_16 = consts.tile([nb, H], BF16)
            nc.vector.tensor_copy(h0_16, h0_sb)
            ht_prev = state.tile([128, KH * nb], MM_DT, tag="HT")
            for n2 in range(2):
                ht0_ps = pt.tile([128, 3 * nb], BF16, tag="hTps")
                for j in range(3):
                    ck = 3 * n2 + j
                    nc.tensor.transpose(
                        ht0_ps[:, j * nb : (j + 1) * nb],
                        h0_16[:, ck * 128 : (ck + 1) * 128],
                        ident16,
                    )
                nc.scalar.copy(
                    out=ht_prev[:, 3 * n2 * nb : (3 * n2 + 3) * nb], in_=ht0_ps
                )

            def stage_x_dma(t0):
                """Issue the DMA for the x block starting at step t0."""
                xstg = xs_pool.tile([128, XBLK, KX, nb], F32, tag="XSTG", name="xstg")
                nblk = min(XBLK, t_steps - t0)
                nc.sync.dma_start(
                    out=xstg[:, 0:nblk],
                    in_=xT_d[t0 : t0 + nblk].rearrange("t k p b -> p t k b"),
                )
                return xstg, nblk

            def convert_x(xstg, nblk):
                """bf16-convert a staged x block (DMA issued well before)."""
                xt = xs_pool.tile([128, XBLK, KX, nb], MM_DT, tag="X", name="xt")
                nc.vector.tensor_copy(xt[:, 0:nblk], xstg[:, 0:nblk])
                return xt

            def emit_x_waves(gates2, x_tile, t):
                """x-only matmul waves (k < KX) for step t, both halves."""
                trel = t % XBLK
                for n2 in range(2):
                    for k in range(KX):
                        for q in range(4):
                            col = q * H + n2 * NHALF
                            nc.tensor.matmul(
                                gates2[n2][32 * q : 32 * q + nb, 0:NHALF],
                                x_tile[:, trel, k, :],
                                w_sb[:, k, col : col + NHALF],
                                start=(k == 0),
                                stop=False,
                                tile_position=(0, 32 * q),
                                skip_group_check=True,
                            )

            def emit_h_waves(gates2, ht, n2):
                """h matmul waves (k in [KX, NK)) for one half."""
                for k in range(KX, NK):
                    ck = k - KX
                    for q in range(4):
                        col = q * H + n2 * NHALF
                        nc.tensor.matmul(
                            gates2[n2][32 * q : 32 * q + nb, 0:NHALF],
                            ht[:, ck * nb : (ck + 1) * nb],
                            u_sb[:, ck, col : col + NHALF],
                            start=False,
                            stop=(k == NK - 1),
                            tile_position=(0, 32 * q),
                            skip_group_check=True,
                        )

            def new_gates():
                return [
                    pg.tile([128, 512], F32, tag="g0", name="gates0", bufs=3),
                    pg.tile([128, 512], F32, tag="g1", name="gates1", bufs=3),
                ]

            # prologue: stage x block 0; emit x-waves for steps 0 and 1
            # (x-waves run two steps ahead so the PE always has
            # recurrence-independent work during the elementwise tail)
            x_tile = convert_x(*stage_x_dma(0))
            gates_cur = new_gates()
            emit_x_waves(gates_cur, x_tile, 0)
            gates_next = new_gates()
            emit_x_waves(gates_next, x_tile, 1)

            for t in range(t_steps):
                # ---- PE: h-waves for this step (both halves) ----
                for n2 in range(2):
                    emit_h_waves(gates_cur, ht_prev, n2)

                gates = gates_cur
                if has_b:
                    for n2 in range(2):
                        for q, base in ((0, 0), (1, 32), (2, 64), (3, 96)):
                            bq = b_sb[:, q * H + n2 * NHALF : q * H + (n2 + 1) * NHALF]
                            bq = bass.AP(
                                tensor=bq.tensor, offset=bq.offset,
                                ap=[[0, nb]] + bq.ap[1:],
                            )
                            nc.vector.tensor_add(
                                gates[n2][base : base + nb, 0:NHALF],
                                gates[n2][base : base + nb, 0:NHALF],
                                bq,
                            )

                # ---- PE: x-waves for step t+2 (independent of h) ----
                gates_cur = gates_next
                if t + 2 < t_steps:
                    # prefetch next block's DMA at mid-block; convert at block edge
                    if (t + 2) % XBLK == XBLK // 2 and t + 2 + XBLK // 2 < t_steps:
                        xstg_pend = stage_x_dma(t + 2 + XBLK // 2)
                    if (t + 2) % XBLK == 0:
                        x_tile = convert_x(*xstg_pend)
                    gates_next = new_gates()
                    emit_x_waves(gates_next, x_tile, t + 2)

                # ---- ACT: sigmoids (one per half; per-row scale does g~) ----
                # S2 rows: i'@0:16, f'@32:48, o'@64:80, sigmoid(2g)@96:112
                S2, C, G, TC, HN = {}, {}, {}, {}, {}
                # (g~ weight columns are pre-doubled on the host, so a plain
                # sigmoid gives sigmoid(2g) on those rows)
                for n2 in range(2):
                    S2[n2] = gsb.tile([112, NHALF], BF16, tag=f"S2{n2}", name=f"S2_{n2}")
                    nc.scalar.activation(
                        out=S2[n2], in_=gates[n2][0:112, 0:NHALF], func=SIG,
                    )

                # ---- DVE chains (half A fully, then half B) ----
                for n2 in range(2):
                    # g~ = 2*sigmoid(2g) - 1  (rows 0:16, matching i' rows)
                    G[n2] = gsb.tile([nb, NHALF], BF16, tag=f"G{n2}", name=f"G_{n2}")
                    nc.vector.tensor_scalar(
                        G[n2], S2[n2][96:112], 2.0, -1.0, MULT, ADD
                    )
                    # i' * g~ (out-shift to rows 32:48 so the add's inputs align)
                    T1 = ew.tile([32 + nb, NHALF], BF16, tag=f"T1{n2}", name=f"T1_{n2}")
                    nc.vector.tensor_mul(T1[32 : 32 + nb], S2[n2][0:nb], G[n2])
                    # f' * c
                    C[n2] = state.tile(
                        [32 + nb, NHALF], BF16, tag=f"C{n2}", name=f"C_{n2}"
                    )
                    nc.vector.tensor_mul(
                        C[n2][32 : 32 + nb], S2[n2][32 : 32 + nb],
                        c_prev[n2][32 : 32 + nb],
                    )
                    nc.vector.tensor_add(
                        C[n2][32 : 32 + nb], C[n2][32 : 32 + nb],
                        T1[32 : 32 + nb],
                    )

                # ---- per half: tanh(c) -> h = o'*tanh(c) -> transpose ->
                # psum->sbuf copy on ACT (DVE stays free for the other
                # half's chain; per-half h^T tiles so next-step waves k=2-4
                # only wait on the A copy) ----
                ht_new = state.tile([128, KH * nb], MM_DT, tag="HT", name="ht_new")
                for n2 in range(2):
                    TC[n2] = ew.tile(
                        [64 + nb, NHALF], BF16, tag=f"TC{n2}", name=f"TC_{n2}"
                    )
                    nc.scalar.activation(
                        out=TC[n2][64 : 64 + nb], in_=C[n2][32 : 32 + nb], func=TANH,
                    )
                    HN[n2] = ew.tile([nb, NHALF], BF16, tag=f"H{n2}", name=f"H_{n2}")
                    nc.vector.tensor_mul(
                        HN[n2], S2[n2][64 : 64 + nb], TC[n2][64 : 64 + nb]
                    )
                    hT_ps = pt.tile([128, 3 * nb], BF16, tag="hTps", name="hT_ps")
                    for j in range(3):
                        nc.tensor.transpose(
                            hT_ps[:, j * nb : (j + 1) * nb],
                            HN[n2][:, j * 128 : (j + 1) * 128],
                            ident16,
                        )
                    nc.vector.tensor_copy(
                        ht_new[:, 3 * n2 * nb : (3 * n2 + 3) * nb], hT_ps
                    )

                for n2 in range(2):
                    nc.sync.dma_start(
                        out=cs_d[:, t, n2], in_=C[n2][32 : 32 + nb]
                    )
                    c_prev[n2] = C[n2]
                nc.sync.dma_start(out=hs_d[t, :, :], in_=ht_new)

                ht_prev = ht_new

    nc.finalize()
    return nc


# Column permutation: reference gate order (i, f, g~, o) -> kernel (i, f, o, g~)
def _gate_perm():
    return np.concatenate(
        [np.arange(0, H), np.arange(H, 2 * H), np.arange(3 * H, 4 * H),
         np.arange(2 * H, 3 * H)]
    )


def _prep_core_inputs(input_, h0, c0, Wp, Up, bp, t_steps):
    nb = input_.shape[0]
    xT = np.ascontiguousarray(
        input_[:, :t_steps].transpose(1, 2, 0).reshape(t_steps, KX, 128, nb)
    )
    return {
        "xT": xT,
        "h0": np.ascontiguousarray(h0),
        "c0": np.ascontiguousarray(
            c0.reshape(nb, 2, NHALF).astype(ml_dtypes.bfloat16)
        ),
        "w": Wp,
        "u": Up,
        "b": bp,
    }


def run(input, hiddenState, cellState, W, U, b, t_steps=T, trace=False):
    input = np.asarray(input, np.float32)
    hiddenState = np.asarray(hiddenState, np.float32)
    cellState = np.asarray(cellState, np.float32)
    W = np.asarray(W, np.float32)
    U = np.asarray(U, np.float32)
    b = np.asarray(b, np.float32)

    perm = _gate_perm()
    # pre-double the g~ gate columns (kernel gate order (i,f,o,g~) -> cols
    # 3H:4H) so the device sigmoid computes sigmoid(2g) with no scale input
    Wp = W[:, perm].copy()
    Up = U[:, perm].copy()
    bp = b[perm].copy()
    Wp[:, 3 * H :] *= 2.0
    Up[:, 3 * H :] *= 2.0
    bp[3 * H :] *= 2.0
    Wp = np.ascontiguousarray(Wp.reshape(KX, 128, 4 * H))
    Up = np.ascontiguousarray(Up.reshape(KH, 128, 4 * H))
    bp = np.ascontiguousarray(bp.reshape(1, 4 * H))
    has_b = bool(np.any(b))

    nc = build_lstm(NB, t_steps, has_b)
    in_maps = []
    for c in range(NCORES):
        bs = slice(c * NB, (c + 1) * NB)
        in_maps.append(
            _prep_core_inputs(
                input[bs], hiddenState[bs], cellState[bs], Wp, Up, bp, t_steps
            )
        )
    res = run_bass_kernel_spmd(
        nc, in_maps, core_ids=list(range(NCORES)), trace=trace
    )

    hs = np.empty((B, t_steps, H), np.float32)
    cs = np.empty((B, t_steps, H), np.float32)
    for c in range(NCORES):
        bs = slice(c * NB, (c + 1) * NB)
        ht = res.results[c]["hs"].astype(np.float32)  # [t, 128, 6*16]
        ht = ht.reshape(t_steps, 128, KH, NB)
        hs[bs] = ht.transpose(3, 0, 2, 1).reshape(NB, t_steps, H)
        cs[bs] = res.results[c]["cs"].astype(np.float32).reshape(NB, t_steps, H)
    return (hs, cs), res


def kernel(input, hiddenState, cellState, W, U, b):
    (hs, cs), _ = run(input, hiddenState, cellState, W, U, b)
    return hs, cs
